# revision 28
# baseline (speedup 1.0000x reference)
"""GCN decoder kernel for Trainium2, 8-core data-parallel over batch.

Per core (one batch sample b):
  Xn = X / max(||X||, 1e-12)                       row-normalize
  S  = Xn @ Xn^T; sig = sigmoid(S - C(1-m_i))      exact-0 masked rows (ACT bias)
  deg = rowsum(sig) - 0.5*n_masked + m;  d = max(deg, 1e-6)^-1/2
  aggT = (m*d)_i * (Y^T @ (sig + diag(m)))  with Y = m*d*X   == (A_norm @ X)^T
  HfT = relu(W1^T aggT + b1);  PT = W2^T HfT + b2
  out = sigmoid(PT^T PT - C(1-m_i)) * m_j          pair-masked output

Pipeline notes (v2):
  - startup processed in 4-jb groups so sigmoid starts ~8us in
  - one [128,2048] ACT per row block (scalar is the S-phase bottleneck:
    (2048+352)/1.2 ns per block)
  - mbc/dbc broadcasts via gpsimd partition_broadcast (PE freed)
  - phase-6 mask multiply split vector/gpsimd; output DMA on 3 queues
"""

from contextlib import ExitStack

import numpy as np

import bass_rust as _bass_rust
import concourse.bass as bass
import concourse.mybir as mybir
import concourse.tile as tile
from concourse import library_config
from concourse.bass_utils import run_bass_kernel_spmd
from concourse.masks import make_identity

F32 = mybir.dt.float32
F16 = mybir.dt.float16
AF = mybir.ActivationFunctionType
OP = mybir.AluOpType

B = 8
N = 2048
D = 256
H = 256
P = 128
NB = N // P  # 16 row blocks
NCH = N // 512  # 4 column chunks of 512
NG = 4  # jb groups of 4
MASK_C = 30000.0


def _install_drain_split(max_waits: int = 1):
    """This walrus build accepts at most ONE sync-wait per instruction.
    (a) split the Tile kernel-tail drain into single-wait drains;
    (b) hoist extra waits from any lowered instruction onto standalone
    EventSemaphore instructions on the same engine."""
    from concourse.vector_clock import ScopedClock

    if getattr(tile.TileContext, "_drain_split_installed", False):
        return

    def _drain_and_barrier(self, tick_clock, wait_clock):
        drain_inst = self.nc.sync.drain()
        wait_clock.add_sem_waits(
            drain_inst.ins, ScopedClock({None: tick_clock.global_clock})
        )
        si = drain_inst.ins.sync_info
        waits = list(si.on_wait) if si is not None and si.on_wait else []
        if len(waits) > max_waits:
            drain_inst.ins.sync_info = _bass_rust.SyncInfo(
                on_wait=waits[:max_waits],
                on_update=list(si.on_update) if si.on_update else [],
            )
            rest = waits[max_waits:]
            for i in range(0, len(rest), max_waits):
                extra = self.nc.sync.drain()
                extra.ins.sync_info = _bass_rust.SyncInfo(
                    on_wait=rest[i : i + max_waits], on_update=[]
                )
        self.nc.all_engine_barrier()
        assert self.sems is not None
        popped = self.nc._tile_sem_poison_stack.pop()
        assert popped is self._sem_poison
        self.nc.clear_and_free_semaphores(list(self.sems.allocated().values()))
        self.nc.all_engine_barrier()

    tile.TileContext._drain_and_barrier = _drain_and_barrier

    orig_add = tile.TileContext._add_instruction
    counter = [0]

    def _add_instruction(self, inst):
        si = inst.sync_info
        if si is not None and si.on_wait and len(si.on_wait) > max_waits:
            waits = list(si.on_wait)
            keep = waits[-max_waits:]
            for w in waits[: -max_waits]:
                counter[0] += 1
                ev = mybir.InstEventSemaphore(
                    name=f"{inst.name}-xw{counter[0]}", ins=[], outs=[]
                )
                ev.engine = inst.engine
                ev.sync_info = _bass_rust.SyncInfo(on_wait=[w], on_update=[])
                orig_add(self, ev)
            inst.sync_info = _bass_rust.SyncInfo(
                on_wait=keep, on_update=list(si.on_update) if si.on_update else []
            )
        orig_add(self, inst)

    tile.TileContext._add_instruction = _add_instruction
    tile.TileContext._drain_split_installed = True


def build_nc(reps=1):
    _install_drain_split()
    nc = bass.Bass("TRN2", target_bir_lowering=False, debug=False, num_devices=B)

    x_d = nc.dram_tensor("x", [N, D], F32, kind="ExternalInput").ap()
    w1_d = nc.dram_tensor("w1", [D, H], F16, kind="ExternalInput").ap()
    w2_d = nc.dram_tensor("w2", [H, H], F16, kind="ExternalInput").ap()
    b1_d = nc.dram_tensor("b1t", [P, H // P], F32, kind="ExternalInput").ap()
    b2_d = nc.dram_tensor("b2t", [P, H // P], F32, kind="ExternalInput").ap()
    mf_d = nc.dram_tensor("mf", [P, NB], F32, kind="ExternalInput").ap()
    rb_d = nc.dram_tensor("rowbias", [P, NB], F32, kind="ExternalInput").ap()
    mr_d = nc.dram_tensor("mrow", [1, N], F16, kind="ExternalInput").ap()
    on_d = nc.dram_tensor("ones16", [1, P], F16, kind="ExternalInput").ap()
    cv_d = nc.dram_tensor("cvec", [P, 1], F32, kind="ExternalInput").ap()
    out_d = nc.dram_tensor("out", [N, N], F32, kind="ExternalOutput").ap()

    with tile.TileContext(nc) as tc:
      for rep in range(reps):
        with ExitStack() as top:
            const = top.enter_context(tc.tile_pool(name=f"const{rep}", bufs=1))
            psum = top.enter_context(
                tc.tile_pool(name=f"psum{rep}", bufs=2, space="PSUM")
            )

            # ---- constants ----
            # early-needed consts go on the scalar queue so the sync queue
            # can start streaming X immediately; w1/w2/b1/b2 (phase-5) are
            # issued on sync after the X loads.
            w1 = const.tile([P, 2, H], F16, tag="w1")  # [d_p, d_chunk, h]
            w2 = const.tile([P, 2, H], F16, tag="w2")
            b1v = const.tile([P, 2], F32, tag="b1v")
            b2v = const.tile([P, 2], F32, tag="b2v")
            mf = const.tile([P, NB], F32, tag="mf")
            nc.scalar.dma_start(mf[:], mf_d[:])
            rb = const.tile([P, NB], F32, tag="rb")
            nc.scalar.dma_start(rb[:], rb_d[:])
            mrow = const.tile([1, N], F16, tag="mrow")
            nc.scalar.dma_start(mrow[:], mr_d[:])
            ones1 = const.tile([1, P], F16, tag="ones1")
            nc.scalar.dma_start(ones1[:], on_d[:])
            cvec = const.tile([P, 1], F32, tag="cvec")
            nc.scalar.dma_start(cvec[:], cv_d[:])
            eye = const.tile([P, P], F32, tag="eye")
            make_identity(nc, eye[:])
            eye16 = const.tile([P, P], F16, tag="eye16")
            make_identity(nc, eye16[:])

            # small per-node vectors in [p, block] layout
            nrm = const.tile([P, NB], F32, tag="nrm")
            dga = const.tile([P, NB], F32, tag="dga")
            dgv = const.tile([P, NB], F32, tag="dgv")
            dpo = const.tile([P, NB], F32, tag="dpo")
            mdv = const.tile([P, NB], F32, tag="mdv")
            sml = const.tile([P, NB], F32, tag="sml")  # scratch for ln
            d16 = const.tile([16, P], F16, tag="d16")
            drow = const.tile([1, N], F16, tag="drow")
            mbc = const.tile([P, N], F16, tag="mbc")  # column mask, bcast
            dbc = const.tile([P, N], F16, tag="dbc")  # (m*d)_i, bcast

            aggt = const.tile([P, 2, N], F16, tag="aggt")

            with ExitStack() as mid:
                xp = mid.enter_context(tc.tile_pool(name=f"xp{rep}", bufs=1))
                xtp = mid.enter_context(tc.tile_pool(name=f"xtp{rep}", bufs=NG))
                xg = [
                    xtp.tile([P, 4, D], F32, tag="x", name=f"x_{rep}_{g}")
                    for g in range(NG)
                ]
                x_sb = [xg[jb // 4][:, jb % 4, :] for jb in range(NB)]
                y16 = xp.tile([P, NB, D], F16, tag="y16")
                xnt = xp.tile([P, 2, N], F16, tag="xnt")
                sig = xp.tile([P, NB, N], F16, tag="sig")
                tmp = mid.enter_context(tc.tile_pool(name=f"tmp{rep}", bufs=4))
                sqp = mid.enter_context(tc.tile_pool(name=f"sqp{rep}", bufs=2))

                # ---- phase 0/1: load X, row norms, Xn^T (fp16) ----
                # processed in groups of 4 row-blocks so the transposes (PE)
                # and the first S matmuls can start early.
                # one 512KB DMA per 4-jb group (amortizes the ~700ns issue
                # cost), alternating the two free queues
                for g in range(NG):
                    eng = nc.sync if g % 2 == 0 else nc.gpsimd
                    eng.dma_start(
                        xg[g][:],
                        x_d[g * 4 * P : (g + 1) * 4 * P, :].rearrange(
                            "(b p) d -> p b d", p=P
                        ),
                    )
                # phase-5 weights after X on the sync queue
                nc.sync.dma_start(w1[:], w1_d.rearrange("(c p) h -> p c h", p=P))
                nc.sync.dma_start(w2[:], w2_d.rearrange("(c p) h -> p c h", p=P))
                nc.sync.dma_start(b1v[:], b1_d[:])
                nc.sync.dma_start(b2v[:], b2_d[:])
                xn_t = [None] * NB
                for g in range(NG):
                    gs = slice(g * 4, g * 4 + 4)
                    # sum of squares: mostly scalar ACT; one per group on
                    # vector (keeps vector free for xn muls + xnt copies)
                    for j in range(4):
                        jb = g * 4 + j
                        if jb % 4 != 1:
                            sq = sqp.tile([P, D], F32, tag="sq")
                            nc.scalar.activation(
                                sq[:],
                                x_sb[jb],
                                AF.Square,
                                accum_out=nrm[:, jb : jb + 1],
                            )
                        else:
                            sq = sqp.tile([P, D], F32, tag="sq")
                            nc.vector.tensor_tensor(
                                sq[:], x_sb[jb], x_sb[jb], op=OP.mult
                            )
                            nc.vector.tensor_reduce(
                                out=nrm[:, jb : jb + 1],
                                in_=sq[:],
                                axis=mybir.AxisListType.X,
                                op=OP.add,
                            )
                    # 1/max(norm,eps) = exp(-0.5*ln(max(nrm,eps^2))), * mask
                    nc.vector.tensor_scalar_max(nrm[:, gs], nrm[:, gs], 1e-24)
                    nc.scalar.activation(sml[:, gs], nrm[:, gs], AF.Ln)
                    nc.scalar.activation(nrm[:, gs], sml[:, gs], AF.Exp, scale=-0.5)
                    nc.vector.tensor_tensor(
                        nrm[:, gs], nrm[:, gs], mf[:, gs], op=OP.mult
                    )
                    # xn (fp16, masked-normalized rows), then PE transposes
                    pt = psum.tile([P, N], F32, tag="big")
                    for j in range(4):
                        jb = g * 4 + j
                        xn = tmp.tile([P, D], F16, tag="xn")
                        xeng = nc.gpsimd if jb % 2 == 1 else nc.vector
                        xeng.tensor_scalar_mul(
                            xn[:], x_sb[jb], nrm[:, jb : jb + 1]
                        )
                        xn_t[jb] = xn
                        for k in range(2):
                            pt16 = pt[:, (2 * j + k) * 64 : (2 * j + k + 1) * 64]
                            nc.tensor.transpose(
                                pt16.bitcast(F16),
                                xn[:, k * P : (k + 1) * P],
                                eye16[:],
                            )
                    # one copy per group: psum slots (j,k) -> xnt[:, k, g*512+j*128]
                    src = (
                        pt[:, 0:512]
                        .bitcast(F16)
                        .rearrange("p (j k f) -> p k j f", j=4, k=2)
                    )
                    dst = xnt[:, :, g * 512 : (g + 1) * 512].rearrange(
                        "p k (j f) -> p k j f", j=4
                    )
                    nc.vector.tensor_copy(out=dst, in_=src)

                # ---- phase 2: S = Xn Xn^T, sigmoid w/ row-mask bias, deg ----
                # deg rowsum: even jb via scalar accum_out, odd jb via a
                # vector reduce over the f16 sigmoid (balances the engines)
                for jb in range(NB):
                    jsl = slice(jb * P, (jb + 1) * P)
                    ps = psum.tile([P, N], F32, tag="big")
                    for k in range(2):
                        for c in range(NCH):
                            csl = slice(c * 512, (c + 1) * 512)
                            nc.tensor.matmul(
                                ps[:, csl],
                                xnt[:, k, jsl],
                                xnt[:, k, csl],
                                start=(k == 0),
                                stop=(k == 1),
                            )
                    if jb % 2 == 0 or jb == NB - 1:
                        # jb 15 must use the scalar accum: a trailing vector
                        # reduce would sit on the deg critical path
                        nc.scalar.activation(
                            sig[:, jb, :],
                            ps[:],
                            AF.Sigmoid,
                            bias=rb[:, jb : jb + 1],
                            accum_out=dga[:, jb : jb + 1],
                        )
                    else:
                        nc.scalar.activation(
                            sig[:, jb, :],
                            ps[:],
                            AF.Sigmoid,
                            bias=rb[:, jb : jb + 1],
                        )
                        nc.vector.tensor_reduce(
                            out=dga[:, jb : jb + 1],
                            in_=sig[:, jb, :],
                            axis=mybir.AxisListType.X,
                            op=OP.add,
                        )
                    # add diag(m) into the diagonal block (after deg reduce)
                    nc.vector.scalar_tensor_tensor(
                        out=sig[:, jb, jsl],
                        in0=eye[:],
                        scalar=mf[:, jb : jb + 1],
                        in1=sig[:, jb, jsl],
                        op0=OP.mult,
                        op1=OP.add,
                    )

                # ---- phase 3: d = max(deg - corr + m, eps)^-1/2, Y, dbc ----
                # dgv = (dga - cvec) + mf   (one vector op)
                nc.vector.scalar_tensor_tensor(
                    out=dgv[:],
                    in0=dga[:],
                    scalar=cvec[:, 0:1],
                    in1=mf[:],
                    op0=OP.subtract,
                    op1=OP.add,
                )
                nc.vector.tensor_scalar_max(dgv[:], dgv[:], 1e-6)
                nc.scalar.activation(sml[:], dgv[:], AF.Ln)
                nc.scalar.activation(dpo[:], sml[:], AF.Exp, scale=-0.5)
                nc.vector.tensor_tensor(mdv[:], mf[:], dpo[:], op=OP.mult)
                # dbc chain first (long pole): transpose mdv, SBUF-gather to a
                # [1, N] row, then rank-1 broadcast matmuls
                ptd = psum.tile([P, N], F32, tag="big")
                nc.tensor.transpose(ptd[0:16, 0:P], mdv[:], eye[:])
                nc.vector.tensor_copy(out=d16[:], in_=ptd[0:16, 0:P])
                nc.sync.dma_start(
                    drow[0:1].rearrange("p (o q) -> p o q", o=16), d16[:]
                )
                # column mask broadcast (needed by phase 6; no DMA dep, and
                # the copy-out runs on the otherwise-idle scalar engine)
                pbm = psum.tile([P, N], F32, tag="big")
                for c in range(NCH):
                    csl = slice(c * 512, (c + 1) * 512)
                    nc.tensor.matmul(
                        pbm[:, csl], ones1[:], mrow[:, csl], start=True, stop=True
                    )
                nc.scalar.activation(mbc[:], pbm[:], AF.Copy)
                # Y = (m*d) * X rows, fp16  (gates phase 4's first matmuls)
                for jb in range(NB):
                    nc.vector.tensor_scalar_mul(
                        y16[:, jb, :], x_sb[jb], mdv[:, jb : jb + 1]
                    )
                pbd = psum.tile([P, N], F32, tag="big")
                for c in range(NCH):
                    csl = slice(c * 512, (c + 1) * 512)
                    nc.tensor.matmul(
                        pbd[:, csl], ones1[:], drow[0:1, csl], start=True, stop=True
                    )
                nc.vector.tensor_copy(out=dbc[:], in_=pbd[:])

                # ---- phase 4: aggT = dbc * (Y^T (sig + diag(m))) ----
                for ig in range(NCH):
                    isl = slice(ig * 512, (ig + 1) * 512)
                    ps = psum.tile([P, N], F32, tag="big")
                    for jb in range(NB):
                        nc.tensor.matmul(
                            ps[:, 0:512],
                            y16[:, jb, 0:P],
                            sig[:, jb, isl],
                            start=(jb == 0),
                            stop=(jb == NB - 1),
                        )
                        nc.tensor.matmul(
                            ps[:, 512:1024],
                            y16[:, jb, P : 2 * P],
                            sig[:, jb, isl],
                            start=(jb == 0),
                            stop=(jb == NB - 1),
                        )
                    nc.vector.tensor_tensor(
                        aggt[:, 0, isl], ps[:, 0:512], dbc[:, isl], op=OP.mult
                    )
                    nc.vector.tensor_tensor(
                        aggt[:, 1, isl], ps[:, 512:1024], dbc[:, isl], op=OP.mult
                    )

            # ---- phase 5: HfT = relu(W1^T aggT + b1), PT = W2^T HfT + b2 ----
            with ExitStack() as bot:
                hp = bot.enter_context(tc.tile_pool(name=f"hp{rep}", bufs=1))
                outp = bot.enter_context(tc.tile_pool(name=f"outp{rep}", bufs=5))
                hft = hp.tile([P, 2, N], F16, tag="hft")
                ptt = hp.tile([P, 2, N], F16, tag="ptt")
                for hb in range(2):
                    hsl = slice(hb * P, (hb + 1) * P)
                    ps = psum.tile([P, N], F32, tag="big")
                    for k in range(2):
                        for c in range(NCH):
                            csl = slice(c * 512, (c + 1) * 512)
                            nc.tensor.matmul(
                                ps[:, csl],
                                w1[:, k, hsl],
                                aggt[:, k, csl],
                                start=(k == 0),
                                stop=(k == 1),
                            )
                    for hh in range(2):
                        hsl2 = slice(hh * 1024, (hh + 1) * 1024)
                        nc.scalar.activation(
                            hft[:, hb, hsl2],
                            ps[:, hsl2],
                            AF.Relu,
                            bias=b1v[:, hb : hb + 1],
                        )
                pps = [
                    psum.tile([P, N], F32, tag="big", name=f"pps{rep}_{hb}")
                    for hb in range(2)
                ]
                for k in range(2):
                    for hb in range(2):
                        hsl = slice(hb * P, (hb + 1) * P)
                        for c in range(NCH):
                            csl = slice(c * 512, (c + 1) * 512)
                            nc.tensor.matmul(
                                pps[hb][:, csl],
                                w2[:, k, hsl],
                                hft[:, k, csl],
                                start=(k == 0),
                                stop=(k == 1),
                            )
                for hb in range(2):
                    for hh in range(2):
                        hsl2 = slice(hh * 1024, (hh + 1) * 1024)
                        nc.scalar.activation(
                            ptt[:, hb, hsl2],
                            pps[hb][:, hsl2],
                            AF.Identity,
                            bias=b2v[:, hb : hb + 1],
                        )

                # ---- phase 6: out = sigmoid(PT^T PT + row bias) * m_j ----
                out_engs = [nc.sync, nc.scalar, nc.gpsimd]
                for jb in range(NB):
                    jsl = slice(jb * P, (jb + 1) * P)
                    ps = psum.tile([P, N], F32, tag="big")
                    for k in range(2):
                        for c in range(NCH):
                            csl = slice(c * 512, (c + 1) * 512)
                            nc.tensor.matmul(
                                ps[:, csl],
                                ptt[:, k, jsl],
                                ptt[:, k, csl],
                                start=(k == 0),
                                stop=(k == 1),
                            )
                    osb = outp.tile([P, N], F32, tag="osb")
                    if jb < NB - 2:
                        nc.scalar.activation(
                            osb[:], ps[:], AF.Sigmoid, bias=rb[:, jb : jb + 1]
                        )
                        nc.vector.tensor_tensor(osb[:], osb[:], mbc[:], op=OP.mult)
                        out_engs[jb % 3].dma_start(out_d[jsl, :], osb[:])
                    else:
                        # last two row blocks in 1024-col chunks to shorten
                        # the ACT->mult->DMA drain tail
                        for c in range(2):
                            csl = slice(c * 1024, (c + 1) * 1024)
                            nc.scalar.activation(
                                osb[:, csl],
                                ps[:, csl],
                                AF.Sigmoid,
                                bias=rb[:, jb : jb + 1],
                            )
                            nc.vector.tensor_tensor(
                                osb[:, csl], osb[:, csl], mbc[:, csl], op=OP.mult
                            )
                            out_engs[(jb + c) % 3].dma_start(
                                out_d[jsl, c * 1024 : (c + 1) * 1024], osb[:, csl]
                            )

    return nc


_NC_CACHE = None


def _get_nc():
    global _NC_CACHE
    if _NC_CACHE is None:
        _NC_CACHE = build_nc()
    return _NC_CACHE


def make_in_maps(X, mask, W1, b1, W2, b2):
    X = np.asarray(X, dtype=np.float32)
    mask = np.asarray(mask)
    W1 = np.asarray(W1, dtype=np.float32)
    b1 = np.asarray(b1, dtype=np.float32)
    W2 = np.asarray(W2, dtype=np.float32)
    b2 = np.asarray(b2, dtype=np.float32)

    b1t = np.ascontiguousarray(b1.reshape(H // P, P).T)
    b2t = np.ascontiguousarray(b2.reshape(H // P, P).T)
    in_maps = []
    for b in range(B):
        m = mask[b].astype(np.float32)
        bias = -MASK_C * (1.0 - m)
        in_maps.append(
            {
                "x": np.ascontiguousarray(X[b]),
                "w1": W1.astype(np.float16),
                "w2": W2.astype(np.float16),
                "b1t": b1t,
                "b2t": b2t,
                "mf": np.ascontiguousarray(m.reshape(NB, P).T),
                "rowbias": np.ascontiguousarray(bias.reshape(NB, P).T),
                "mrow": m.reshape(1, N).astype(np.float16),
                "ones16": np.ones((1, P), dtype=np.float16),
                "cvec": np.full((P, 1), 0.5 * float(N - m.sum()), dtype=np.float32),
            }
        )
    return in_maps


def kernel(X, mask, W1, b1, W2, b2):
    nc = _get_nc()
    in_maps = make_in_maps(X, mask, W1, b1, W2, b2)
    res = run_bass_kernel_spmd(nc, in_maps, list(range(B)))
    out = np.stack([res.results[b]["out"] for b in range(B)], axis=0)
    return out.astype(np.float32)
